# revision 5
# baseline (speedup 1.0000x reference)
"""Trainium2 Bass kernel for a (buggy-but-well-defined) ConvTranspose2d.

Math (matches the reference exactly):
  out[b, co, i, j] = sum_{ci,kh,kw} ker[ci,co,3-kh,3-kw] * xpad[b,ci,i+kh,j+kw]
                     + bias_sum * cnt[i] * cnt[j]          for i,j in [0,66)
  out is zero elsewhere in the (B,128,126,126) output.
  xpad = x[:, :, :63, :63] zero-padded by 3 on every side.
  cnt  = conv(ones(63), ones(4)) = [1,2,3,4,...,4,3,2,1]  (len 66)

Strategy: data-parallel over batch (2 items / core on 8 cores), bf16.
Per core, per image, 10 groups of <=7 output rows; each group accumulates
its 16 shifted 128x128xN matmuls (contraction over ci) into one PSUM bank.
The PE stream is pure bf16 (1 col/cycle, fast FWL weight loads, no
fp32-mode switches).  x is shipped with horizontal padding only; each
group's first matmul (a full-row-coverage tap) covers the whole PSUM
region with start=True, and every other tap is trimmed to the rows AND
columns that touch real data (per-element PSUM has_written bits make the
partial accumulation well-defined).  The bias field, replicated across
all 128 partitions, is shipped from the host and fused into the
PSUM-drain as one DVE tensor_add per group (PSUM + bias -> bf16 out
tile).  DMA issue is spread over three engines (sync: weights + output,
scalar: image chunks, gpsimd: bias field) so descriptor-issue latency
doesn't serialize the startup.  The mostly-zero full output is assembled
host-side.
"""

import ml_dtypes
import numpy as np

import concourse.bacc as bacc
import concourse.mybir as mybir
import concourse.tile as tile
from concourse.bass_utils import run_bass_kernel_spmd

B, CIN, COUT, K, H, W = 16, 128, 128, 4, 64, 64
NCORES = 8
BPC = B // NCORES          # batch items per core
HV = H - 1                 # 63 valid input rows/cols
RS = HV + 2 * (K - 1)      # 69: row stride (cols padded by 3 each side)
HO = HV + K - 1            # 66 output rows/cols (nonzero region)
HOUT = (H - 1) * 2         # 126 full output rows/cols
NWT = K * K * COUT         # 2048 weight cols
NBR = HO * HO              # 4356 replicated-bias cols
NXI = HV * RS              # 4347 cols per image (63 rows x 69 padded cols)
NXW = NWT + NBR + BPC * NXI
F32 = mybir.dt.float32
BF16 = mybir.dt.bfloat16

GROUPS = [(0, 7), (7, 7), (14, 7), (21, 7), (28, 5),
          (33, 7), (40, 7), (47, 7), (54, 7), (61, 5)]

# Tap layout order in the weight tensor: kh=3 first so group 0 (whose
# full-coverage tap is kh=3) can start on the first small weight chunk.
KH_LAYOUT = [3, 0, 1, 2]
TAP_COL = {}
for _i, _kh in enumerate(KH_LAYOUT):
    for _kw in range(K):
        TAP_COL[(_kh, _kw)] = (_i * K + _kw) * COUT

B0_CHUNKS = [(0, 7), (7, 16), (23, 16), (39, 24)]
B1_CHUNKS = [(0, 32), (32, 31)]

NWARM = 3                  # junk warm-up matmuls (HAM clock-gate)

_CACHE = {}


def _kh_order(i0, r):
    """Tap row order for a group: a full-row-coverage kh first (its kw=0
    matmul carries start=True and must clear the whole PSUM region)."""
    def full(kh):
        return 3 - kh - i0 <= 0 and 66 - kh - i0 >= r
    return sorted(range(K), key=lambda kh: not full(kh))


def _build_nc():
    # Bacc (not raw Bass): its finalize() legalizes sync waits — moving
    # excess matmul waits onto LDWEIGHTS and splitting multi-waits onto
    # EventSemaphore instructions — which walrus codegen requires.
    nc = bacc.Bacc(None)
    xw = nc.dram_tensor("xw", [CIN, NXW], BF16, kind="ExternalInput")
    out = nc.dram_tensor("out", [BPC, COUT, HO, HO], BF16,
                         kind="ExternalOutput")

    with tile.TileContext(nc) as tc:
        with (
            tc.tile_pool(name="xwpool", bufs=1) as xwpool,
            tc.tile_pool(name="warm", bufs=1) as warmpool,
            tc.tile_pool(name="wps", bufs=1, space="PSUM") as warmpsum,
            tc.tile_pool(name="acc", bufs=4, space="PSUM") as psum_pool,
            tc.tile_pool(name="opool", bufs=4) as opool,
        ):
            # PE warm-up fodder: zeros, no DMA dependency.
            warmt = warmpool.tile([CIN, 462], BF16)
            nc.gpsimd.memset(warmt, 0.0)

            xwt = xwpool.tile([CIN, NXW], BF16)
            brt = xwt[:, NWT:NWT + NBR]
            xv = xwt[:, NWT + NBR:].rearrange("p (b r c) -> p b r c",
                                              b=BPC, r=HV, c=RS)
            xwsrc = xw[:, NWT + NBR:].rearrange("p (b r c) -> p b r c",
                                                b=BPC, r=HV, c=RS)

            # All input DMAs on ONE queue (scalar), in exact consumption
            # order: per-queue FIFO means each transfer gets the full DMA
            # bandwidth and earlier-needed bytes always land first.  Sync
            # keeps only the output DMAs.
            def xchunk(b, r0, nr):
                nc.scalar.dma_start(xv[:, b, r0:r0 + nr, :],
                                    xwsrc[:, b, r0:r0 + nr, :])

            nc.scalar.dma_start(xwt[:, :8 * COUT], xw[:, :8 * COUT])
            xchunk(0, *B0_CHUNKS[0])
            nc.scalar.dma_start(xwt[:, 8 * COUT:NWT], xw[:, 8 * COUT:NWT])
            xchunk(0, *B0_CHUNKS[1])
            nc.scalar.dma_start(brt, xw[:, NWT:NWT + NBR])
            for c in B0_CHUNKS[2:]:
                xchunk(0, *c)
            for c in B1_CHUNKS:
                xchunk(1, *c)

            # Junk matmuls: nudge the HAM clock gate while DMA lands.
            wps = warmpsum.tile([COUT, 462], F32)
            for _ in range(NWARM):
                nc.tensor.matmul(wps, warmt[:, :CIN], warmt[:, :462],
                                 start=True, stop=True)

            # Main conv stream: groups outer, taps inner.  The first tap
            # (full row coverage, kw=0) writes the whole PSUM region with
            # start=True; all other taps are trimmed to real-data rows
            # (di0:di1) and columns (63 of 66) and accumulate into a 2D
            # row/col window of the bank.
            for b in range(BPC):
                for i0, r in GROUPS:
                    acc = psum_pool.tile([COUT, 462], F32,
                                         tag="acc", name="acc")
                    av = acc.rearrange("p (r c) -> p r c", r=7, c=HO)
                    order = _kh_order(i0, r)
                    for kh in order:
                        di0 = max(0, 3 - kh - i0)
                        di1 = min(r, 66 - kh - i0)
                        x0 = i0 + kh - 3 + di0
                        for kw in range(K):
                            lhsT = xwt[:, TAP_COL[(kh, kw)]:
                                       TAP_COL[(kh, kw)] + COUT]
                            first = kh == order[0] and kw == 0
                            last = kh == order[-1] and kw == K - 1
                            if first:
                                rhs = xv[:, b, x0:x0 + r, 0:HO]
                                dst = acc[:, :r * HO]
                            else:
                                c0 = max(0, 3 - kw)
                                rhs = xv[:, b, x0:x0 + di1 - di0, 3:3 + HV]
                                dst = av[:, di0:di1, c0:c0 + HV]
                            nc.tensor.matmul(dst, lhsT, rhs,
                                             start=first, stop=last)
                    otile = opool.tile([COUT, 462], BF16, tag="ot", name="ot")
                    nc.vector.tensor_add(otile[:, :r * HO], acc[:, :r * HO],
                                         brt[:, i0 * HO:(i0 + r) * HO])
                    nc.sync.dma_start(out[b, :, i0:i0 + r, :],
                                      otile[:, :r * HO])
    nc.finalize()
    return nc


def get_nc():
    if "nc" not in _CACHE:
        _CACHE["nc"] = _build_nc()
    return _CACHE["nc"]


def prep_inputs(x, kernel, bias):
    """Host-side prep: per-core input maps (numpy only, negligible cost)."""
    x = np.asarray(x, dtype=np.float32)
    ker = np.asarray(kernel, dtype=np.float32)
    bias = np.asarray(bias, dtype=np.float32)

    kf = ker[:COUT, :, ::-1, ::-1]                    # [ci, co, kh, kw] flipped
    wt = np.empty((CIN, NWT), ml_dtypes.bfloat16)
    for kh in range(K):
        for kw in range(K):
            c = TAP_COL[(kh, kw)]
            wt[:, c:c + COUT] = kf[:, :, kh, kw].astype(ml_dtypes.bfloat16)

    cnt = np.convolve(np.ones(HV, np.float32), np.ones(K, np.float32))
    bias_sum = np.sum(bias[:COUT], dtype=np.float32)
    bfield = (bias_sum * np.outer(cnt, cnt)).astype(ml_dtypes.bfloat16).ravel()

    xb = x[:, :, :HV, :HV].astype(ml_dtypes.bfloat16)
    in_maps = []
    for c in range(NCORES):
        xwm = np.zeros((CIN, NXW), ml_dtypes.bfloat16)
        xwm[:, :NWT] = wt
        xwm[:, NWT:NWT + NBR] = bfield[None, :]
        xp = xwm[:, NWT + NBR:].reshape(CIN, BPC, HV, RS)
        xp[:, :, :, K - 1:K - 1 + HV] = \
            xb[c * BPC:(c + 1) * BPC].transpose(1, 0, 2, 3)
        in_maps.append({"xw": xwm})
    return in_maps


def assemble(per_core_outs):
    out = np.zeros((B, COUT, HOUT, HOUT), np.float32)
    for c, o in enumerate(per_core_outs):
        out[c * BPC:(c + 1) * BPC, :, :HO, :HO] = np.asarray(o, np.float32)
    return out


def run(inputs, **spmd_kwargs):
    """Returns (full_output, BassKernelResults)."""
    nc = get_nc()
    in_maps = prep_inputs(**inputs)
    res = run_bass_kernel_spmd(nc, in_maps, list(range(NCORES)), **spmd_kwargs)
    return assemble([r["out"] for r in res.results]), res


def kernel(**inputs):
    out, _ = run(inputs)
    return out


# revision 8
# speedup vs baseline: 1.1725x; 1.1725x over previous
"""Trainium2 Bass kernel for a (buggy-but-well-defined) ConvTranspose2d.

Math (matches the reference exactly):
  out[b, co, i, j] = sum_{ci,kh,kw} ker[ci,co,3-kh,3-kw] * xpad[b,ci,i+kh,j+kw]
                     + bias_sum * cnt[i] * cnt[j]          for i,j in [0,66)
  out is zero elsewhere in the (B,128,126,126) output.
  xpad = x[:, :, :63, :63] zero-padded by 3 on every side.
  cnt  = conv(ones(63), ones(4)) = [1,2,3,4,...,4,3,2,1]  (len 66)

Strategy: data-parallel over batch (2 items / core on 8 cores), bf16.
Per core, per image, 10 groups of <=7 output rows; each group accumulates
its 16 shifted 128x128xN matmuls (contraction over ci) into one PSUM bank.
The PE stream is pure bf16 (1 col/cycle, fast FWL weight loads, no
fp32-mode switches).  x is shipped with horizontal padding only; each
group's first matmul (a full-row-coverage tap) covers the whole PSUM
region with start=True, and every other tap is trimmed to the rows AND
columns that touch real data (per-element PSUM has_written bits make the
partial accumulation well-defined).  The bias field, replicated across
all 128 partitions, is shipped from the host and fused into the
PSUM-drain as one DVE tensor_add per group (PSUM + bias -> bf16 out
tile).  DMA issue is spread over three engines (sync: weights + output,
scalar: image chunks, gpsimd: bias field) so descriptor-issue latency
doesn't serialize the startup.  The mostly-zero full output is assembled
host-side.
"""

import ml_dtypes
import numpy as np

import concourse.bacc as bacc
import concourse.mybir as mybir
import concourse.tile as tile
from concourse.bass_utils import run_bass_kernel_spmd

B, CIN, COUT, K, H, W = 16, 128, 128, 4, 64, 64
NCORES = 8
BPC = B // NCORES          # batch items per core
HV = H - 1                 # 63 valid input rows/cols
RS = HV + 2 * (K - 1)      # 69: row stride (cols padded by 3 each side)
HO = HV + K - 1            # 66 output rows/cols (nonzero region)
HOUT = (H - 1) * 2         # 126 full output rows/cols
NWT = K * K * COUT         # 2048 weight cols
NBR = HO * HO              # 4356 replicated-bias cols
NXI = HV * RS              # 4347 cols per image (63 rows x 69 padded cols)
NXW = NWT + NBR + BPC * NXI
F32 = mybir.dt.float32
BF16 = mybir.dt.bfloat16

GROUPS = [(0, 7), (7, 7), (14, 7), (21, 7), (28, 5),
          (33, 7), (40, 7), (47, 7), (54, 7), (61, 5)]

# Tap layout order in the weight tensor: kh=3 first so group 0 (whose
# full-coverage tap is kh=3) can start on the first small weight chunk.
KH_LAYOUT = [3, 0, 1, 2]
TAP_COL = {}
for _i, _kh in enumerate(KH_LAYOUT):
    for _kw in range(K):
        TAP_COL[(_kh, _kw)] = (_i * K + _kw) * COUT

NWARM = 4                  # junk warm-up matmuls (HAM clock-gate)

_CACHE = {}


def _kh_order(i0, r):
    """Tap row order for a group: a full-row-coverage kh first (its kw=0
    matmul carries start=True and must clear the whole PSUM region)."""
    def full(kh):
        return 3 - kh - i0 <= 0 and 66 - kh - i0 >= r
    return sorted(range(K), key=lambda kh: not full(kh))


def _build_nc():
    # Bacc (not raw Bass): its finalize() legalizes sync waits — moving
    # excess matmul waits onto LDWEIGHTS and splitting multi-waits onto
    # EventSemaphore instructions — which walrus codegen requires.
    nc = bacc.Bacc(None)
    xw = nc.dram_tensor("xw", [CIN, NXW], BF16, kind="ExternalInput")
    out = nc.dram_tensor("out", [BPC, COUT, HO, HO], BF16,
                         kind="ExternalOutput")

    with tile.TileContext(nc) as tc:
        with (
            tc.tile_pool(name="xwpool", bufs=1) as xwpool,
            tc.tile_pool(name="warm", bufs=1) as warmpool,
            tc.tile_pool(name="wps", bufs=1, space="PSUM") as warmpsum,
            tc.tile_pool(name="acc", bufs=5, space="PSUM") as psum_pool,
            tc.tile_pool(name="opool", bufs=4) as opool,
        ):
            # PE warm-up fodder: zeros, no DMA dependency.
            warmt = warmpool.tile([CIN, 462], BF16)
            nc.gpsimd.memset(warmt, 0.0)

            xwt = xwpool.tile([CIN, NXW], BF16)
            brt = xwt[:, NWT:NWT + NBR]
            xv = xwt[:, NWT + NBR:].rearrange("p (b r c) -> p b r c",
                                              b=BPC, r=HV, c=RS)
            xwsrc = xw[:, NWT + NBR:].rearrange("p (b r c) -> p b r c",
                                                b=BPC, r=HV, c=RS)

            # All input DMAs on ONE queue (scalar) in exact consumption
            # order (per-queue FIFO = strict priority); kept coarse so
            # per-partition lines stay multi-KB (DMA packet efficiency).
            # Sync keeps only the output DMAs.
            def xchunk(b, r0, nr):
                nc.scalar.dma_start(xv[:, b, r0:r0 + nr, :],
                                    xwsrc[:, b, r0:r0 + nr, :])

            def brchunk(r0, nr):
                nc.scalar.dma_start(brt[:, r0 * HO:(r0 + nr) * HO],
                                    xw[:, NWT + r0 * HO:NWT + (r0 + nr) * HO])

            nc.scalar.dma_start(xwt[:, :4 * COUT], xw[:, :4 * COUT])
            xchunk(0, 0, 7)
            nc.scalar.dma_start(xwt[:, 4 * COUT:NWT], xw[:, 4 * COUT:NWT])
            xchunk(0, 7, 24)
            brchunk(0, 33)
            xchunk(0, 31, 32)
            brchunk(33, 33)
            xchunk(1, 0, 32)
            xchunk(1, 32, 31)

            # Junk matmuls: nudge the HAM clock gate while DMA lands.
            wps = warmpsum.tile([COUT, 462], F32)
            for _ in range(NWARM):
                nc.tensor.matmul(wps, warmt[:, :CIN], warmt[:, :462],
                                 start=True, stop=True)

            # Main conv stream: groups outer, taps inner.  The first tap
            # (full row coverage, kw=0) writes the whole PSUM region with
            # start=True; all other taps are trimmed to real-data rows
            # (di0:di1) and columns (63 of 66) and accumulate into a 2D
            # row/col window of the bank.
            for b in range(BPC):
                for i0, r in GROUPS:
                    acc = psum_pool.tile([COUT, 462], F32,
                                         tag="acc", name="acc")
                    av = acc.rearrange("p (r c) -> p r c", r=7, c=HO)
                    order = _kh_order(i0, r)
                    for kh in order:
                        di0 = max(0, 3 - kh - i0)
                        di1 = min(r, 66 - kh - i0)
                        x0 = i0 + kh - 3 + di0
                        for kw in range(K):
                            lhsT = xwt[:, TAP_COL[(kh, kw)]:
                                       TAP_COL[(kh, kw)] + COUT]
                            first = kh == order[0] and kw == 0
                            last = kh == order[-1] and kw == K - 1
                            if first:
                                rhs = xv[:, b, x0:x0 + r, 0:HO]
                                dst = acc[:, :r * HO]
                            else:
                                c0 = max(0, 3 - kw)
                                rhs = xv[:, b, x0:x0 + di1 - di0, 3:3 + HV]
                                dst = av[:, di0:di1, c0:c0 + HV]
                            nc.tensor.matmul(dst, lhsT, rhs,
                                             start=first, stop=last)
                    otile = opool.tile([COUT, 462], BF16, tag="ot", name="ot")
                    nc.vector.tensor_add(otile[:, :r * HO], acc[:, :r * HO],
                                         brt[:, i0 * HO:(i0 + r) * HO])
                    nc.sync.dma_start(out[b, :, i0:i0 + r, :],
                                      otile[:, :r * HO])
    nc.finalize()
    return nc


def get_nc():
    if "nc" not in _CACHE:
        _CACHE["nc"] = _build_nc()
    return _CACHE["nc"]


def prep_inputs(x, kernel, bias):
    """Host-side prep: per-core input maps (numpy only, negligible cost)."""
    x = np.asarray(x, dtype=np.float32)
    ker = np.asarray(kernel, dtype=np.float32)
    bias = np.asarray(bias, dtype=np.float32)

    kf = ker[:COUT, :, ::-1, ::-1]                    # [ci, co, kh, kw] flipped
    wt = np.empty((CIN, NWT), ml_dtypes.bfloat16)
    for kh in range(K):
        for kw in range(K):
            c = TAP_COL[(kh, kw)]
            wt[:, c:c + COUT] = kf[:, :, kh, kw].astype(ml_dtypes.bfloat16)

    cnt = np.convolve(np.ones(HV, np.float32), np.ones(K, np.float32))
    bias_sum = np.sum(bias[:COUT], dtype=np.float32)
    bfield = (bias_sum * np.outer(cnt, cnt)).astype(ml_dtypes.bfloat16).ravel()

    xb = x[:, :, :HV, :HV].astype(ml_dtypes.bfloat16)
    in_maps = []
    for c in range(NCORES):
        xwm = np.zeros((CIN, NXW), ml_dtypes.bfloat16)
        xwm[:, :NWT] = wt
        xwm[:, NWT:NWT + NBR] = bfield[None, :]
        xp = xwm[:, NWT + NBR:].reshape(CIN, BPC, HV, RS)
        xp[:, :, :, K - 1:K - 1 + HV] = \
            xb[c * BPC:(c + 1) * BPC].transpose(1, 0, 2, 3)
        in_maps.append({"xw": xwm})
    return in_maps


def assemble(per_core_outs):
    out = np.zeros((B, COUT, HOUT, HOUT), np.float32)
    for c, o in enumerate(per_core_outs):
        out[c * BPC:(c + 1) * BPC, :, :HO, :HO] = np.asarray(o, np.float32)
    return out


def run(inputs, **spmd_kwargs):
    """Returns (full_output, BassKernelResults)."""
    nc = get_nc()
    in_maps = prep_inputs(**inputs)
    res = run_bass_kernel_spmd(nc, in_maps, list(range(NCORES)), **spmd_kwargs)
    return assemble([r["out"] for r in res.results]), res


def kernel(**inputs):
    out, _ = run(inputs)
    return out


# revision 11
# speedup vs baseline: 1.1944x; 1.0187x over previous
"""Trainium2 Bass kernel for a (buggy-but-well-defined) ConvTranspose2d.

Math (matches the reference exactly):
  out[b, co, i, j] = sum_{ci,kh,kw} ker[ci,co,3-kh,3-kw] * xpad[b,ci,i+kh,j+kw]
                     + bias_sum * cnt[i] * cnt[j]          for i,j in [0,66)
  out is zero elsewhere in the (B,128,126,126) output.
  xpad = x[:, :, :63, :63] zero-padded by 3 on every side.
  cnt  = conv(ones(63), ones(4)) = [1,2,3,4,...,4,3,2,1]  (len 66)

Strategy: data-parallel over batch (2 items / core on 8 cores), bf16.
Per core, per image, 10 groups of <=7 output rows; each group accumulates
its 16 shifted 128x128xN matmuls (contraction over ci) into one PSUM bank.
The PE stream is pure bf16 (1 col/cycle, fast FWL weight loads, no
fp32-mode switches).  x is shipped with horizontal padding only; each
group's first matmul (a full-row-coverage tap) covers the whole PSUM
region with start=True, and every other tap is trimmed to the rows AND
columns that touch real data (per-element PSUM has_written bits make the
partial accumulation well-defined).  The bias field, replicated across
all 128 partitions, is shipped from the host and fused into the
PSUM-drain as one DVE tensor_add per group (PSUM + bias -> bf16 out
tile).  DMA issue is spread over three engines (sync: weights + output,
scalar: image chunks, gpsimd: bias field) so descriptor-issue latency
doesn't serialize the startup.  The mostly-zero full output is assembled
host-side.
"""

import ml_dtypes
import numpy as np

import concourse.bacc as bacc
import concourse.mybir as mybir
import concourse.tile as tile
from concourse.bass_utils import run_bass_kernel_spmd

B, CIN, COUT, K, H, W = 16, 128, 128, 4, 64, 64
NCORES = 8
BPC = B // NCORES          # batch items per core
HV = H - 1                 # 63 valid input rows/cols
RS = HV + 2 * (K - 1)      # 69: row stride (cols padded by 3 each side)
HO = HV + K - 1            # 66 output rows/cols (nonzero region)
HOUT = (H - 1) * 2         # 126 full output rows/cols
NWT = K * K * COUT         # 2048 weight cols
NBR = HO * HO              # 4356 replicated-bias cols
NXI = HV * RS              # 4347 cols per image (63 rows x 69 padded cols)
NXW = NWT + NBR + BPC * NXI
F32 = mybir.dt.float32
BF16 = mybir.dt.bfloat16

GROUPS = [(0, 7), (7, 7), (14, 7), (21, 7), (28, 5),
          (33, 7), (40, 7), (47, 7), (54, 7), (61, 5)]

# Tap layout order in the weight tensor: kh=3 first so group 0 (whose
# full-coverage tap is kh=3) can start on the first small weight chunk.
KH_LAYOUT = [3, 0, 1, 2]
TAP_COL = {}
for _i, _kh in enumerate(KH_LAYOUT):
    for _kw in range(K):
        TAP_COL[(_kh, _kw)] = (_i * K + _kw) * COUT

NWARM = 10                 # junk warm-up matmuls (HAM clock-gate)

_CACHE = {}


def _kh_order(i0, r):
    """Tap row order for a group: a full-row-coverage kh first (its kw=0
    matmul carries start=True and must clear the whole PSUM region)."""
    def full(kh):
        return 3 - kh - i0 <= 0 and 66 - kh - i0 >= r
    return sorted(range(K), key=lambda kh: not full(kh))


def _build_nc():
    # Bacc (not raw Bass): its finalize() legalizes sync waits — moving
    # excess matmul waits onto LDWEIGHTS and splitting multi-waits onto
    # EventSemaphore instructions — which walrus codegen requires.
    nc = bacc.Bacc(None)
    xw = nc.dram_tensor("xw", [CIN, NXW], BF16, kind="ExternalInput")
    out = nc.dram_tensor("out", [BPC, COUT, HO, HO], BF16,
                         kind="ExternalOutput")

    with tile.TileContext(nc) as tc:
        with (
            tc.tile_pool(name="xwpool", bufs=1) as xwpool,
            tc.tile_pool(name="warm", bufs=1) as warmpool,
            tc.tile_pool(name="wps", bufs=1, space="PSUM") as warmpsum,
            tc.tile_pool(name="acc", bufs=5, space="PSUM") as psum_pool,
            tc.tile_pool(name="opool", bufs=4) as opool,
        ):
            # PE warm-up fodder: zeros, no DMA dependency.
            warmt = warmpool.tile([CIN, 462], BF16)
            nc.vector.memset(warmt, 0.0)

            xwt = xwpool.tile([CIN, NXW], BF16)
            brt = xwt[:, NWT:NWT + NBR]
            xv = xwt[:, NWT + NBR:].rearrange("p (b r c) -> p b r c",
                                              b=BPC, r=HV, c=RS)
            xwsrc = xw[:, NWT + NBR:].rearrange("p (b r c) -> p b r c",
                                                b=BPC, r=HV, c=RS)

            # All input DMAs on ONE queue (scalar) in exact consumption
            # order (per-queue FIFO = strict priority); kept coarse so
            # per-partition lines stay multi-KB (DMA packet efficiency).
            # Sync keeps only the output DMAs.
            def xchunk(b, r0, nr):
                nc.scalar.dma_start(xv[:, b, r0:r0 + nr, :],
                                    xwsrc[:, b, r0:r0 + nr, :])

            def brchunk(r0, nr):
                nc.scalar.dma_start(brt[:, r0 * HO:(r0 + nr) * HO],
                                    xw[:, NWT + r0 * HO:NWT + (r0 + nr) * HO])

            nc.scalar.dma_start(xwt[:, :4 * COUT], xw[:, :4 * COUT])
            xchunk(0, 0, 7)
            nc.scalar.dma_start(xwt[:, 4 * COUT:NWT], xw[:, 4 * COUT:NWT])
            xchunk(0, 7, 14)
            brchunk(0, 66)
            xchunk(0, 21, 42)
            xchunk(1, 0, 63)

            # Junk matmuls: nudge the HAM clock gate while DMA lands.
            wps = warmpsum.tile([COUT, 462], F32)
            for _ in range(NWARM):
                nc.tensor.matmul(wps, warmt[:, :CIN], warmt[:, :462],
                                 start=True, stop=True)

            # Main conv stream: groups outer, taps inner.  The first tap
            # (full row coverage, kw=0) writes the whole PSUM region with
            # start=True; all other taps are trimmed to real-data rows
            # (di0:di1) and columns (63 of 66) and accumulate into a 2D
            # row/col window of the bank.
            for b in range(BPC):
                for i0, r in GROUPS:
                    acc = psum_pool.tile([COUT, 462], F32,
                                         tag="acc", name="acc")
                    av = acc.rearrange("p (r c) -> p r c", r=7, c=HO)
                    order = _kh_order(i0, r)
                    for kh in order:
                        di0 = max(0, 3 - kh - i0)
                        di1 = min(r, 66 - kh - i0)
                        x0 = i0 + kh - 3 + di0
                        for kw in range(K):
                            lhsT = xwt[:, TAP_COL[(kh, kw)]:
                                       TAP_COL[(kh, kw)] + COUT]
                            first = kh == order[0] and kw == 0
                            last = kh == order[-1] and kw == K - 1
                            if first:
                                rhs = xv[:, b, x0:x0 + r, 0:HO]
                                dst = acc[:, :r * HO]
                            else:
                                c0 = max(0, 3 - kw)
                                rhs = xv[:, b, x0:x0 + di1 - di0, 3:3 + HV]
                                dst = av[:, di0:di1, c0:c0 + HV]
                            nc.tensor.matmul(dst, lhsT, rhs,
                                             start=first, stop=last)
                    otile = opool.tile([COUT, 462], BF16, tag="ot", name="ot")
                    nc.vector.tensor_add(otile[:, :r * HO], acc[:, :r * HO],
                                         brt[:, i0 * HO:(i0 + r) * HO])
                    nc.sync.dma_start(out[b, :, i0:i0 + r, :],
                                      otile[:, :r * HO])
    nc.finalize()
    return nc


def get_nc():
    if "nc" not in _CACHE:
        _CACHE["nc"] = _build_nc()
    return _CACHE["nc"]


def prep_inputs(x, kernel, bias):
    """Host-side prep: per-core input maps (numpy only, negligible cost)."""
    x = np.asarray(x, dtype=np.float32)
    ker = np.asarray(kernel, dtype=np.float32)
    bias = np.asarray(bias, dtype=np.float32)

    kf = ker[:COUT, :, ::-1, ::-1]                    # [ci, co, kh, kw] flipped
    wt = np.empty((CIN, NWT), ml_dtypes.bfloat16)
    for kh in range(K):
        for kw in range(K):
            c = TAP_COL[(kh, kw)]
            wt[:, c:c + COUT] = kf[:, :, kh, kw].astype(ml_dtypes.bfloat16)

    cnt = np.convolve(np.ones(HV, np.float32), np.ones(K, np.float32))
    bias_sum = np.sum(bias[:COUT], dtype=np.float32)
    bfield = (bias_sum * np.outer(cnt, cnt)).astype(ml_dtypes.bfloat16).ravel()

    xb = x[:, :, :HV, :HV].astype(ml_dtypes.bfloat16)
    in_maps = []
    for c in range(NCORES):
        xwm = np.zeros((CIN, NXW), ml_dtypes.bfloat16)
        xwm[:, :NWT] = wt
        xwm[:, NWT:NWT + NBR] = bfield[None, :]
        xp = xwm[:, NWT + NBR:].reshape(CIN, BPC, HV, RS)
        xp[:, :, :, K - 1:K - 1 + HV] = \
            xb[c * BPC:(c + 1) * BPC].transpose(1, 0, 2, 3)
        in_maps.append({"xw": xwm})
    return in_maps


def assemble(per_core_outs):
    out = np.zeros((B, COUT, HOUT, HOUT), np.float32)
    for c, o in enumerate(per_core_outs):
        out[c * BPC:(c + 1) * BPC, :, :HO, :HO] = np.asarray(o, np.float32)
    return out


def run(inputs, **spmd_kwargs):
    """Returns (full_output, BassKernelResults)."""
    nc = get_nc()
    in_maps = prep_inputs(**inputs)
    res = run_bass_kernel_spmd(nc, in_maps, list(range(NCORES)), **spmd_kwargs)
    return assemble([r["out"] for r in res.results]), res


def kernel(**inputs):
    out, _ = run(inputs)
    return out
